# revision 4
# baseline (speedup 1.0000x reference)
"""Trainium2 Bass kernel for MultiHeadAttention (RoPE + GQA + causal) on 8 NeuronCores.

Reference model: B=2, T=2048, C=2048, 16 heads (HD=128), 4 KV heads, RoPE, causal.

Sharding: core c = 4*b + g handles batch b and KV group g (4 query heads + 1 KV head).
Each core computes a partial output x[b] @ (its head slice of the attention) @ Wo-slice;
the host sums the 4 partials per batch.

All matmuls run in float32r (fp32 data, ~12 mantissa bits in the PE, bf16-class speed).
Data flow is "transposed" (head-dim on partitions) so that:
  - projections:  qT/kT/vT[d, t] = W[c, d].T @ xT[c, t]     (xT built by PE transposes)
  - scores:       S^T[k, q] = kT[:, kt].T @ qT[:, q]         (no transpose of P needed)
  - probs:        P^T = exp(S^T * scale) on ScalarE, diag tile masked by tril multiply
  - y:            yT[d, q] += vnat[kt].T @ P^T[kt]           (vnat = PE-transposed vT)
  - denominators: sums[*, q] += ones.T @ P^T[kt]             (broadcast over partitions)
  - out:          out[t, c] += yT[:, h, t].T @ Wo[h]         (normalized y)
RoPE uses de-interleaved head dims ([re(0:64) | im(64:128)], via host-permuted W
columns) so the rotation is two aligned multiplies plus one cross-partition swap.
"""

import numpy as np

import concourse.bacc as bacc
import concourse.mybir as mybir
import concourse.tile as tile
from concourse.bass import ts
from concourse.bass_utils import run_bass_kernel_spmd
from concourse.masks import make_identity

F32 = mybir.dt.float32
F32R = mybir.dt.float32r
Copy = mybir.ActivationFunctionType.Copy
Exp = mybir.ActivationFunctionType.Exp

SCALE = 0.08838834764831845  # 1/sqrt(128)


def build_program(T=2048, C=2048, NH=4, HD=128):
    """Build the per-core SPMD program. Returns compiled Bacc."""
    CT = C // 128        # contraction tiles
    TT = T // 128        # t tiles
    SBW = 512            # superblock width (queries per attention block)
    NSB = T // SBW       # superblocks
    D = NH * HD          # local q width (512)
    CCW = 256            # out-projection column chunk width
    NCC = C // CCW

    nc = bacc.Bacc("TRN2", target_bir_lowering=False, debug=False)

    x_d = nc.dram_tensor("x", [T, C], F32, kind="ExternalInput")
    wq_d = nc.dram_tensor("wq", [C, D], F32, kind="ExternalInput")
    wk_d = nc.dram_tensor("wk", [C, HD], F32, kind="ExternalInput")
    wv_d = nc.dram_tensor("wv", [C, HD], F32, kind="ExternalInput")
    wo_d = nc.dram_tensor("wo", [D, C], F32, kind="ExternalInput")
    cs_d = nc.dram_tensor("cs", [128, T], F32, kind="ExternalInput")
    sn_d = nc.dram_tensor("sn", [128, T], F32, kind="ExternalInput")
    tril_d = nc.dram_tensor("tril", [128, 128], F32, kind="ExternalInput")
    out_d = nc.dram_tensor("out", [T, C], F32, kind="ExternalOutput")

    with tile.TileContext(nc) as tc:
        with (
            tc.tile_pool(name="pw", bufs=1) as pw,
            tc.tile_pool(name="pst", bufs=1) as pst,
            tc.tile_pool(name="pc2", bufs=2) as pc2,
            tc.tile_pool(name="pq", bufs=3) as pq,
            tc.tile_pool(name="pp", bufs=4) as pp,
            tc.tile_pool(name="pr", bufs=2) as pr,
            tc.tile_pool(name="po", bufs=3) as po,
            tc.tile_pool(name="psA", bufs=2, space="PSUM") as psA,
            tc.tile_pool(name="psS", bufs=3, space="PSUM") as psS,
            tc.tile_pool(name="psY", bufs=1, space="PSUM") as psY,
            tc.tile_pool(name="psZ", bufs=1, space="PSUM") as psZ,
        ):
            # ---- constants ----
            ident_f = pw.tile([128, 128], F32, tag="idf")
            make_identity(nc, ident_f[:])
            ident_r = pw.tile([128, 128], F32R, tag="idr")
            nc.vector.tensor_copy(ident_r[:], ident_f[:])
            ones_f = pw.tile([128, 128], F32, tag="onf")
            nc.gpsimd.memset(ones_f[:], 1.0)
            ones_r = pw.tile([128, 128], F32R, tag="onr")
            nc.vector.tensor_copy(ones_r[:], ones_f[:])
            tril_t = pw.tile([128, 128], F32, tag="tri")
            nc.sync.dma_start(tril_t[:], tril_d.ap())

            # ---- weights (load f32, round to f32r) ----
            wq_r = pw.tile([128, CT, D], F32R, tag="wq")
            wq_ap = wq_d.ap().rearrange("(ct p) d -> p ct d", p=128)
            qtr = max(1, CT // 4)
            for i in range(0, CT, qtr):
                st = pst.tile([128, qtr, D], F32, tag="stage")
                nc.sync.dma_start(st[:], wq_ap[:, i : i + qtr, :])
                nc.vector.tensor_copy(wq_r[:, i : i + qtr, :], st[:])
            wk_r = pw.tile([128, CT, HD], F32R, tag="wk")
            wv_r = pw.tile([128, CT, HD], F32R, tag="wv")
            for w_r, w_d in ((wk_r, wk_d), (wv_r, wv_d)):
                st = pst.tile([128, CT, HD], F32, tag="stage")
                nc.sync.dma_start(st[:], w_d.ap().rearrange("(ct p) d -> p ct d", p=128))
                nc.vector.tensor_copy(w_r[:], st[:])

            # ---- persistent activations ----
            xT = pw.tile([128, CT, SBW], F32R, tag="xT")       # per-sb x^T
            kT = pw.tile([128, T], F32R, tag="kT")
            vnat = pw.tile([128, TT, HD], F32R, tag="vnat")
            yT = pw.tile([128, NH, T], F32R, tag="yT")

            def rope(ps, out_ap, cs_t, sn_t):
                a = pr.tile([128, SBW], F32, tag="ra")
                nc.vector.tensor_mul(a[:], ps[:], cs_t[:])
                sw = pr.tile([128, SBW], F32, tag="rs")
                nc.vector.tensor_copy(sw[0:64, :], ps[64:128, :])
                nc.vector.tensor_copy(sw[64:128, :], ps[0:64, :])
                b = pr.tile([128, SBW], F32, tag="rb")
                nc.vector.tensor_mul(b[:], sw[:], sn_t[:])
                nc.vector.tensor_add(out_ap, a[:], b[:])

            for J in range(NSB):
                tsl = ts(J, SBW)
                cs_t = pc2.tile([128, SBW], F32, tag="cs")
                sn_t = pc2.tile([128, SBW], F32, tag="sn")
                nc.sync.dma_start(cs_t[:], cs_d.ap()[:, tsl])
                nc.sync.dma_start(sn_t[:], sn_d.ap()[:, tsl])

                # -- x^T for this superblock (PE transposes of 128x128 blocks) --
                for pair in range(SBW // 256):
                    xn = pst.tile([128, 2, C], F32, tag="xnat")
                    r0 = J * SBW + pair * 256
                    nc.sync.dma_start(
                        xn[:], x_d.ap()[r0 : r0 + 256, :].rearrange("(tt p) c -> p tt c", p=128)
                    )
                    for ct in range(CT):
                        ps = psA.tile([128, 256], F32, tag="mm")
                        for jj in range(2):
                            nc.tensor.transpose(
                                ps[:, ts(jj, 128)], xn[:, jj, ts(ct, 128)], ident_f[:]
                            )
                        nc.scalar.activation(
                            xT[:, ct, pair * 256 : pair * 256 + 256], ps[:], Copy
                        )

                # -- projections --
                qhs = []
                for h in range(NH):
                    ps = psA.tile([128, SBW], F32, tag="mm")
                    for ct in range(CT):
                        nc.tensor.matmul(
                            ps[:], wq_r[:, ct, ts(h, HD)], xT[:, ct, :],
                            start=(ct == 0), stop=(ct == CT - 1),
                        )
                    qh = pq.tile([128, SBW], F32R, tag="qh")
                    rope(ps, qh[:], cs_t, sn_t)
                    qhs.append(qh)

                ps = psA.tile([128, SBW], F32, tag="mm")
                for ct in range(CT):
                    nc.tensor.matmul(
                        ps[:], wk_r[:, ct, :], xT[:, ct, :],
                        start=(ct == 0), stop=(ct == CT - 1),
                    )
                rope(ps, kT[:, tsl], cs_t, sn_t)

                ps = psA.tile([128, SBW], F32, tag="mm")
                for ct in range(CT):
                    nc.tensor.matmul(
                        ps[:], wv_r[:, ct, :], xT[:, ct, :],
                        start=(ct == 0), stop=(ct == CT - 1),
                    )
                vtsb = pc2.tile([128, SBW], F32R, tag="vtsb")
                nc.scalar.activation(vtsb[:], ps[:], Copy)
                for jj in range(SBW // 128):
                    tp = psA.tile([128, 128], F32R, tag="mm")
                    nc.tensor.transpose(tp[:], vtsb[:, ts(jj, 128)], ident_r[:])
                    nc.scalar.activation(vnat[:, J * (SBW // 128) + jj, :], tp[:], Copy)

                # -- attention for each head --
                for h in range(NH):
                    qh = qhs[h]
                    kts = (J + 1) * (SBW // 128)
                    yac = psY.tile([128, SBW], F32, tag="yac")
                    sac = psZ.tile([128, SBW], F32, tag="sac")
                    pts = [None] * kts

                    def do_s(kt, qh=qh, J=J, pts=pts):
                        S = psS.tile([128, SBW], F32, tag="S")
                        nc.tensor.matmul(
                            S[:], kT[:, ts(kt, 128)], qh[:], start=True, stop=True
                        )
                        pt = pp.tile([128, SBW], F32R, tag="pt")
                        jr = kt - J * (SBW // 128)
                        c0 = 128 * jr if jr > 0 else 0
                        nc.scalar.activation(pt[:, c0:], S[:, c0:], Exp, scale=SCALE)
                        if jr >= 0:
                            dsl = slice(128 * jr, 128 * jr + 128)
                            nc.vector.tensor_mul(pt[:, dsl], pt[:, dsl], tril_t[:])
                        pts[kt] = (pt, c0)

                    do_s(0)
                    if kts > 1:
                        do_s(1)
                    for kt in range(kts):
                        if kt + 2 < kts:
                            do_s(kt + 2)
                        pt, c0 = pts[kt]
                        nc.tensor.matmul(
                            yac[:, c0:], vnat[:, kt, :], pt[:, c0:],
                            start=(kt == 0), stop=(kt == kts - 1),
                        )
                        nc.tensor.matmul(
                            sac[:, c0:], ones_r[:], pt[:, c0:],
                            start=(kt == 0), stop=(kt == kts - 1),
                        )
                    recip = pc2.tile([128, SBW], F32, tag="recip")
                    nc.vector.reciprocal(recip[:], sac[:])
                    nc.vector.tensor_mul(yT[:, h, tsl], yac[:], recip[:])

            # ---- output projection ----
            wo_ap = wo_d.ap().rearrange("(hd p) c -> p hd c", p=128)
            for cc in range(NCC):
                st = pst.tile([128, NH, CCW], F32, tag="stage")
                nc.sync.dma_start(st[:], wo_ap[:, :, ts(cc, CCW)])
                woc = pc2.tile([128, NH, CCW], F32R, tag="woc")
                nc.vector.tensor_copy(woc[:], st[:])
                for tt in range(TT):
                    ps = psA.tile([128, CCW], F32, tag="mm")
                    for hd in range(NH):
                        nc.tensor.matmul(
                            ps[:], yT[:, hd, ts(tt, 128)], woc[:, hd, :],
                            start=(hd == 0), stop=(hd == NH - 1),
                        )
                    ot = po.tile([128, CCW], F32, tag="ost")
                    nc.scalar.activation(ot[:], ps[:], Copy)
                    nc.sync.dma_start(out_d.ap()[ts(tt, 128), ts(cc, CCW)], ot[:])

    nc.compile()
    return nc


def make_host_inputs(x, Wq, Wk, Wv, Wo, freqs_cos, freqs_sin, n_kv=4, rep=4, hd=128):
    """Per-core input maps. Core c = 4*b + g -> batch b, kv group g."""
    T = x.shape[1]
    perm = np.concatenate([np.arange(0, hd, 2), np.arange(1, hd, 2)])
    cs = np.empty((hd, T), np.float32)
    cs[: hd // 2] = freqs_cos.T
    cs[hd // 2 :] = freqs_cos.T
    sn = np.empty((hd, T), np.float32)
    sn[: hd // 2] = -freqs_sin.T
    sn[hd // 2 :] = freqs_sin.T
    tril = np.triu(np.ones((128, 128), np.float32))

    in_maps = []
    for c in range(x.shape[0] * n_kv):
        b, g = divmod(c, n_kv)
        cols_q = np.concatenate([(g * rep + hl) * hd + perm for hl in range(rep)])
        in_maps.append({
            "x": np.ascontiguousarray(x[b]),
            "wq": np.ascontiguousarray(Wq[:, cols_q]),
            "wk": np.ascontiguousarray(Wk[:, g * hd + perm]),
            "wv": np.ascontiguousarray(Wv[:, g * hd : (g + 1) * hd]),
            "wo": np.ascontiguousarray(Wo[g * rep * hd : (g + 1) * rep * hd, :]),
            "cs": cs,
            "sn": sn,
            "tril": tril,
        })
    return in_maps


_cache = {}


def kernel(x, Wq, Wk, Wv, Wo, freqs_cos, freqs_sin):
    x = np.asarray(x, dtype=np.float32)
    Wq = np.asarray(Wq, dtype=np.float32)
    Wk = np.asarray(Wk, dtype=np.float32)
    Wv = np.asarray(Wv, dtype=np.float32)
    Wo = np.asarray(Wo, dtype=np.float32)
    freqs_cos = np.asarray(freqs_cos, dtype=np.float32)
    freqs_sin = np.asarray(freqs_sin, dtype=np.float32)

    if "full" not in _cache:
        _cache["full"] = build_program(T=x.shape[1], C=x.shape[2])
    nc = _cache["full"]

    in_maps = make_host_inputs(x, Wq, Wk, Wv, Wo, freqs_cos, freqs_sin)
    res = run_bass_kernel_spmd(nc, in_maps, list(range(8)))
    outs = [res.results[c]["out"] for c in range(8)]

    B, T, C = x.shape
    y = np.empty((B, T, C), np.float32)
    for b in range(B):
        y[b] = outs[4 * b] + outs[4 * b + 1] + outs[4 * b + 2] + outs[4 * b + 3]
    return y


# revision 12
# speedup vs baseline: 1.0153x; 1.0153x over previous
"""Trainium2 Bass kernel for MultiHeadAttention (RoPE + GQA + causal) on 8 NeuronCores.

Reference model: B=2, T=2048, C=2048, 16 heads (HD=128), 4 KV heads, RoPE, causal.

Sharding: core c = 4*b + g handles batch b and KV group g (4 query heads + 1 KV head).
Each core computes the partial output x[b] @ (its head slice of attention) @ Wo-slice;
the host sums the 4 partials per batch.

All matmuls run in float32r (fp32 storage, ~12 explicit mantissa bits in the PE,
bf16-class speed). Matmul operands are pre-rounded to the f32r grid on the HOST, so
DMA loads are legal f32r producers and no on-chip rounding passes are needed.

Data flow keeps head-dim on partitions ("transposed") so that:
  - projections:  qT/kT/vT[d, t] = W[c, d].T @ xT[c, t]      (xT pre-transposed on host)
  - scores:       S^T[k, q] = kT[:, kt].T @ qT[:, q]          (no transpose of P needed)
  - probs:        P^T = exp(S^T * scale) on ScalarE; causal handled by narrowed
                  matmuls plus one triu multiply on the diagonal 128x128 block
  - y:            yT[d, q] += vnat[kt].T @ P^T[kt]            (vnat = PE-transposed vT)
  - denominators: sums[*, q] += ones.T @ P^T[kt]              (PE broadcast over partitions)
  - normalize:    yT *= 1/sums (DVE reciprocal + multiply)
  - out:          out[t, c] += yT[:, h, t].T @ Wo[h]          (block layout, host reassembles)
RoPE uses de-interleaved head dims ([re(0:64) | im(64:128)] via host-permuted W columns):
rotation = two aligned multiplies + one cross-partition swap copy on VectorE.
"""

import numpy as np

import concourse.bacc as bacc
import concourse.mybir as mybir
import concourse.tile as tile
from concourse.bass import ts
from concourse.bass_utils import run_bass_kernel_spmd
from concourse.masks import make_identity

F32 = mybir.dt.float32
F32R = mybir.dt.float32r
Copy = mybir.ActivationFunctionType.Copy
Exp = mybir.ActivationFunctionType.Exp

SCALE = 0.08838834764831845  # 1/sqrt(128)


def round_f32r(x):
    """Round fp32 to the PE's f32r grid (12 explicit mantissa bits, RNE)."""
    u = np.ascontiguousarray(x, dtype=np.float32).view(np.uint32).copy()
    lsb = ((u >> np.uint32(11)) & np.uint32(1)).astype(np.uint32)
    u += np.uint32(0x3FF) + lsb
    u &= np.uint32(0xFFFFF800)
    return u.view(np.float32)


def build_program(T=2048, C=2048, NH=4, HD=128):
    """Build the per-core SPMD program. Returns compiled Bacc."""
    CT = C // 128        # contraction tiles
    TT = T // 128        # t tiles
    SBW = 512            # superblock width (queries per attention block)
    NSB = T // SBW       # superblocks
    D = NH * HD          # local q width (512)
    CCW = 512            # out-projection column chunk width
    NCC = C // CCW

    nc = bacc.Bacc("TRN2", target_bir_lowering=False, debug=False)

    xt_d = nc.dram_tensor("xt", [C, T], F32R, kind="ExternalInput")    # x^T, pre-rounded
    wq_d = nc.dram_tensor("wq", [C, D], F32R, kind="ExternalInput")    # pre-rounded
    wk_d = nc.dram_tensor("wk", [C, HD], F32R, kind="ExternalInput")
    wv_d = nc.dram_tensor("wv", [C, HD], F32R, kind="ExternalInput")
    wo_d = nc.dram_tensor("wo", [D, C], F32R, kind="ExternalInput")
    cs_d = nc.dram_tensor("cs", [128, T], F32, kind="ExternalInput")
    sn_d = nc.dram_tensor("sn", [128, T], F32, kind="ExternalInput")
    tril_d = nc.dram_tensor("tril", [128, 128], F32, kind="ExternalInput")
    out_d = nc.dram_tensor("out", [NCC, TT, 128, CCW], F32, kind="ExternalOutput")

    xt_ap = xt_d.ap().rearrange("(ct p) t -> p ct t", p=128)

    with tile.TileContext(nc) as tc:
        with (
            tc.tile_pool(name="pw", bufs=1) as pw,
            tc.tile_pool(name="pc2", bufs=2) as pc2,
            tc.tile_pool(name="pc1", bufs=1) as pc1,
            tc.tile_pool(name="pq", bufs=3) as pq,
            tc.tile_pool(name="pp", bufs=6) as pp,
            tc.tile_pool(name="pr", bufs=1) as pr,
            tc.tile_pool(name="po", bufs=4) as po,
            tc.tile_pool(name="px", bufs=1) as px,
            tc.tile_pool(name="psA", bufs=3, space="PSUM") as psA,
            tc.tile_pool(name="psS", bufs=3, space="PSUM") as psS,
            tc.tile_pool(name="psY", bufs=1, space="PSUM") as psY,
            tc.tile_pool(name="psZ", bufs=1, space="PSUM") as psZ,
        ):
            # ---- constants ----
            ident_f = pw.tile([128, 128], F32, tag="idf")
            make_identity(nc, ident_f[:])
            ident_r = pw.tile([128, 128], F32R, tag="idr")
            nc.vector.tensor_copy(ident_r[:], ident_f[:])
            ones_f = pw.tile([128, 128], F32, tag="onf")
            nc.gpsimd.memset(ones_f[:], 1.0)
            ones_r = pw.tile([128, 128], F32R, tag="onr")
            nc.vector.tensor_copy(ones_r[:], ones_f[:])
            tril_t = pw.tile([128, 128], F32, tag="tri")
            nc.sync.dma_start(tril_t[:], tril_d.ap())

            # ---- weights (pre-rounded f32r, direct loads) ----
            wq_r = pw.tile([128, CT, D], F32R, tag="wq")
            nc.sync.dma_start(wq_r[:], wq_d.ap().rearrange("(ct p) d -> p ct d", p=128))
            wk_r = pw.tile([128, CT, HD], F32R, tag="wk")
            nc.sync.dma_start(wk_r[:], wk_d.ap().rearrange("(ct p) d -> p ct d", p=128))
            wv_r = pw.tile([128, CT, HD], F32R, tag="wv")
            nc.sync.dma_start(wv_r[:], wv_d.ap().rearrange("(ct p) d -> p ct d", p=128))

            # ---- persistent activations ----
            kT = pw.tile([128, T], F32R, tag="kT")
            vnat = pw.tile([128, TT, HD], F32R, tag="vnat")
            yT = pw.tile([128, NH, T], F32R, tag="yT")

            def rope(ps, out_ap, cs_t, sn_t):
                a = pr.tile([128, SBW], F32, tag="ra")
                nc.vector.tensor_mul(a[:], ps[:], cs_t[:])
                sw = pr.tile([128, SBW], F32, tag="rs")
                nc.vector.tensor_copy(sw[0:64, :], ps[64:128, :])
                nc.vector.tensor_copy(sw[64:128, :], ps[0:64, :])
                nc.vector.tensor_mul(sw[:], sw[:], sn_t[:])
                nc.vector.tensor_add(out_ap, a[:], sw[:])

            for J in range(NSB):
                tsl = ts(J, SBW)
                cs_t = pc1.tile([128, SBW], F32, tag="cs")
                sn_t = pc1.tile([128, SBW], F32, tag="sn")
                nc.sync.dma_start(cs_t[:], cs_d.ap()[:, tsl])
                nc.sync.dma_start(sn_t[:], sn_d.ap()[:, tsl])

                # -- x^T slab for this superblock (direct strided load) --
                xT = px.tile([128, CT, SBW], F32R, tag="xT")
                nc.sync.dma_start(xT[:], xt_ap[:, :, tsl])

                # -- k/v projections --
                ps = psA.tile([128, SBW], F32, tag="mm")
                for ct in range(CT):
                    nc.tensor.matmul(
                        ps[:], wk_r[:, ct, :], xT[:, ct, :],
                        start=(ct == 0), stop=(ct == CT - 1),
                    )
                rope(ps, kT[:, tsl], cs_t, sn_t)

                ps = psA.tile([128, SBW], F32, tag="mm")
                for ct in range(CT):
                    nc.tensor.matmul(
                        ps[:], wv_r[:, ct, :], xT[:, ct, :],
                        start=(ct == 0), stop=(ct == CT - 1),
                    )
                vtsb = pc2.tile([128, SBW], F32R, tag="vtsb")
                nc.vector.tensor_copy(vtsb[:], ps[:])
                for jj in range(SBW // 128):
                    tp = psA.tile([128, 128], F32R, tag="mm")
                    nc.tensor.transpose(tp[:], vtsb[:, ts(jj, 128)], ident_r[:])
                    nc.vector.tensor_copy(vnat[:, J * (SBW // 128) + jj, :], tp[:])

                # -- per head: q projection then attention --
                for h in range(NH):
                    ps = psA.tile([128, SBW], F32, tag="mm")
                    for ct in range(CT):
                        nc.tensor.matmul(
                            ps[:], wq_r[:, ct, ts(h, HD)], xT[:, ct, :],
                            start=(ct == 0), stop=(ct == CT - 1),
                        )
                    qh = pq.tile([128, SBW], F32R, tag="qh")
                    rope(ps, qh[:], cs_t, sn_t)
                    kts = (J + 1) * (SBW // 128)
                    yac = psY.tile([128, SBW], F32, tag="yac")
                    sac = psZ.tile([128, SBW], F32, tag="sac")
                    pts = [None] * kts

                    def do_s(kt, qh=qh, J=J, pts=pts):
                        S = psS.tile([128, SBW], F32, tag="S")
                        nc.tensor.matmul(
                            S[:], kT[:, ts(kt, 128)], qh[:], start=True, stop=True
                        )
                        pt = pp.tile([128, SBW], F32R, tag="pt")
                        jr = kt - J * (SBW // 128)
                        c0 = 128 * jr if jr > 0 else 0
                        nc.scalar.activation(pt[:, c0:], S[:, c0:], Exp, scale=SCALE)
                        if jr >= 0:
                            dsl = slice(128 * jr, 128 * jr + 128)
                            nc.vector.tensor_mul(pt[:, dsl], pt[:, dsl], tril_t[:])
                        pts[kt] = (pt, c0)

                    do_s(0)
                    if kts > 1:
                        do_s(1)
                    for kt in range(kts):
                        if kt + 2 < kts:
                            do_s(kt + 2)
                        pt, c0 = pts[kt]
                        nc.tensor.matmul(
                            yac[:, c0:], vnat[:, kt, :], pt[:, c0:],
                            start=(kt == 0), stop=(kt == kts - 1),
                        )
                        nc.tensor.matmul(
                            sac[:, c0:], ones_r[:], pt[:, c0:],
                            start=(kt == 0), stop=(kt == kts - 1),
                        )
                    recip = pc1.tile([128, SBW], F32, tag="recip")
                    nc.vector.reciprocal(recip[:], sac[:])
                    nc.vector.tensor_mul(yT[:, h, tsl], yac[:], recip[:])

            # ---- output projection (block-layout stores, host reassembles) ----
            wo_ap = wo_d.ap().rearrange("(hd p) c -> p hd c", p=128)
            for cc in range(NCC):
                woc = pc2.tile([128, NH, CCW], F32R, tag="woc")
                nc.sync.dma_start(woc[:], wo_ap[:, :, ts(cc, CCW)])
                for tt in range(TT):
                    ps = psA.tile([128, CCW], F32, tag="mm")
                    for hd in range(NH):
                        nc.tensor.matmul(
                            ps[:], yT[:, hd, ts(tt, 128)], woc[:, hd, :],
                            start=(hd == 0), stop=(hd == NH - 1),
                        )
                    ot = po.tile([128, CCW], F32, tag="ost")
                    nc.vector.tensor_copy(ot[:], ps[:])
                    nc.sync.dma_start(out_d.ap()[cc, tt], ot[:])

    nc.compile()
    return nc


def assemble_out(arr, T, C):
    """Block-layout device output [C//512, T//128, 128, 512] -> [T, C]."""
    return arr.transpose(1, 2, 0, 3).reshape(T, C)


def make_host_inputs(x, Wq, Wk, Wv, Wo, freqs_cos, freqs_sin, n_kv=4, rep=4, hd=128):
    """Per-core input maps. Core c = 4*b + g -> batch b, kv group g."""
    T = x.shape[1]
    perm = np.concatenate([np.arange(0, hd, 2), np.arange(1, hd, 2)])
    cs = np.empty((hd, T), np.float32)
    cs[: hd // 2] = freqs_cos.T
    cs[hd // 2 :] = freqs_cos.T
    sn = np.empty((hd, T), np.float32)
    sn[: hd // 2] = -freqs_sin.T
    sn[hd // 2 :] = freqs_sin.T
    tril = np.triu(np.ones((128, 128), np.float32))

    xt_by_batch = [np.ascontiguousarray(round_f32r(x[b]).T) for b in range(x.shape[0])]
    wq_r, wk_r, wv_r, wo_r = (round_f32r(w) for w in (Wq, Wk, Wv, Wo))

    in_maps = []
    for c in range(x.shape[0] * n_kv):
        b, g = divmod(c, n_kv)
        cols_q = np.concatenate([(g * rep + hl) * hd + perm for hl in range(rep)])
        in_maps.append({
            "xt": xt_by_batch[b],
            "wq": np.ascontiguousarray(wq_r[:, cols_q]),
            "wk": np.ascontiguousarray(wk_r[:, g * hd + perm]),
            "wv": np.ascontiguousarray(wv_r[:, g * hd : (g + 1) * hd]),
            "wo": np.ascontiguousarray(wo_r[g * rep * hd : (g + 1) * rep * hd, :]),
            "cs": cs,
            "sn": sn,
            "tril": tril,
        })
    return in_maps


_cache = {}


def kernel(x, Wq, Wk, Wv, Wo, freqs_cos, freqs_sin):
    x = np.asarray(x, dtype=np.float32)
    Wq = np.asarray(Wq, dtype=np.float32)
    Wk = np.asarray(Wk, dtype=np.float32)
    Wv = np.asarray(Wv, dtype=np.float32)
    Wo = np.asarray(Wo, dtype=np.float32)
    freqs_cos = np.asarray(freqs_cos, dtype=np.float32)
    freqs_sin = np.asarray(freqs_sin, dtype=np.float32)

    if "full" not in _cache:
        _cache["full"] = build_program(T=x.shape[1], C=x.shape[2])
    nc = _cache["full"]

    in_maps = make_host_inputs(x, Wq, Wk, Wv, Wo, freqs_cos, freqs_sin)
    res = run_bass_kernel_spmd(nc, in_maps, list(range(8)))

    B, T, C = x.shape
    outs = [assemble_out(res.results[c]["out"], T, C) for c in range(8)]
    y = np.empty((B, T, C), np.float32)
    for b in range(B):
        y[b] = outs[4 * b] + outs[4 * b + 1] + outs[4 * b + 2] + outs[4 * b + 3]
    return y
